# revision 19
# baseline (speedup 1.0000x reference)
"""Trainium2 Bass kernel for nn_DynamicCNN (hypernetwork conv stack).

Self-contained: hardcodes all shapes/sharding. Strategy:
  - Phase A (per-core, batch-sharded): k/v token MLPs + masked softmax pooling
    -> q_queries for the core's 2 samples. AllGather -> qq_all [32, 512].
  - Phase B (hypernet, column-sharded): each core holds a 16504-column slice
    of the concatenated gf_W (weights read once chip-wide instead of 8x).
    pvec_slice [32, 16504] -> AllToAll -> per-sample full pvec.
  - Phase C (per-core, batch-sharded): 9-layer dynamic conv stack in SBUF,
    channels-on-partitions layout; pairwise-concat handled as two accumulating
    matmuls with +/-1 column offsets (zero data movement).
  - Classifier: AllGather a_rep -> whole-batch BatchNorm MLP, replicated.
"""
import os
from contextlib import ExitStack

import numpy as np

import concourse.bass as bass
import concourse.mybir as mybir
import concourse.tile as tile
from concourse import bacc
from concourse.bass_utils import run_bass_kernel_spmd

# ---------------- constants ----------------
B, H, Q, D, F, Lq, La = 16, 2, 512, 512, 128, 512, 512
NLAYERS = 9
E_LIST = [128, 64, 32, 16, 16, 16, 16, 16, 16]
IN_DIMS = [512] + [2 * E_LIST[i - 1] for i in range(1, NLAYERS)]
EPS = 1e-5
NC = 8
SPC = B // NC          # samples per core
RPC = SPC * H          # bh rows per core
SLICE_COLS = 16504     # gf_W columns per core
HQ = H * Q             # 1024
TOK = SPC * Lq         # tokens per core
C1 = 2 * Q * H         # 2048 hidden
P = 128
FP32 = mybir.dt.float32


def _layer_offsets():
    offs, off = [], 0
    for L in range(NLAYERS):
        Dp, E = IN_DIMS[L], E_LIST[L]
        o = {"w": off, "w_len": Dp * E // NC}
        off += o["w_len"]
        o["b"] = off; o["b_len"] = E // NC
        off += o["b_len"]
        o["fw"] = off; o["fw_len"] = E * F // NC
        off += o["fw_len"]
        o["fb"] = off; o["fb_len"] = F // NC
        off += o["fb_len"]
        offs.append(o)
    assert off == SLICE_COLS
    return offs


LAYER_OFF = _layer_offsets()


# ---------------- device program ----------------
def build_program(debug=False):
    nc = bacc.Bacc("TRN2", target_bir_lowering=False, debug=False, num_devices=NC)
    dt = FP32
    AF = mybir.ActivationFunctionType
    ALU = mybir.AluOpType

    def din(name, shape):
        return nc.dram_tensor(name, shape, dt, kind="ExternalInput").ap()

    qeT = din("qeT", [D, TOK])
    ahT = din("ahT", [D, TOK])
    qmf = din("qmf", [1, TOK])            # 0/1 float mask over tokens
    amf = din("amf", [SPC, La])
    kW1 = din("kW1", [D, C1]); kb1 = din("kb1", [P, C1 // P])
    kW2 = din("kW2", [C1, HQ]); kb2 = din("kb2", [P, HQ // P])
    vW1 = din("vW1", [D, C1]); vb1 = din("vb1", [P, C1 // P])
    vW2 = din("vW2", [C1, HQ]); vb2 = din("vb2", [P, HQ // P])
    gfW = din("gfW", [D, SLICE_COLS])
    gfb = din("gfb", [1, SLICE_COLS])
    lngneg = din("lngneg", [P, NLAYERS])  # NEGATED ln gamma (packed per layer)
    lnb = din("lnb", [P, NLAYERS])
    cW1 = din("cW1", [H * F, 4 * P]); cb1 = din("cb1", [P, 4])
    cg1 = din("cg1", [P, 4]); cbt1 = din("cbt1", [P, 4])
    cW2 = din("cW2", [4 * P, 4 * P]); cb2 = din("cb2", [P, 4])
    cg2 = din("cg2", [P, 4]); cbt2 = din("cbt2", [P, 4])
    cW3 = din("cW3", [4 * P, 4]); cb3 = din("cb3", [4, 1])

    out_ap = nc.dram_tensor("out", [B, 4], dt, kind="ExternalOutput").ap()
    dbg = {}
    if debug:
        dbg["qq"] = nc.dram_tensor("dbg_qq", [4 * NC, Q], dt, kind="ExternalOutput").ap()
        dbg["pvec"] = nc.dram_tensor("dbg_pvec", [4 * NC, SLICE_COLS], dt, kind="ExternalOutput").ap()
        dbg["arep"] = nc.dram_tensor("dbg_arep", [4 * NC, F], dt, kind="ExternalOutput").ap()

    groups = [list(range(NC))]

    with tile.TileContext(nc) as tc, ExitStack() as ctx:
        dram = ctx.enter_context(tc.tile_pool(name="dram", bufs=1, space="DRAM"))
        ag_in = dram.tile([RPC, Q], dt)
        qq_all = dram.tile([4 * NC, Q], dt, addr_space="Shared")
        a2a_in = dram.tile([4 * NC, SLICE_COLS], dt)
        a2a_out = dram.tile([4 * NC, SLICE_COLS], dt)
        arep_in = dram.tile([RPC, F], dt)
        arep_all = dram.tile([4 * NC, F], dt, addr_space="Shared")

        consts = ctx.enter_context(tc.tile_pool(name="consts", bufs=1))
        persist = ctx.enter_context(tc.tile_pool(name="persist", bufs=1))

        # persistent small tensors
        qm_bc = persist.tile([P, TOK], dt)
        nc.sync.dma_start(qm_bc[:], qmf[0:1, :].to_broadcast([P, TOK]))
        am_bc = []
        for s in range(SPC):
            t = persist.tile([P, La], dt, tag=f"am_bc{s}")
            nc.sync.dma_start(t[:], amf[s:s + 1, :].to_broadcast([P, La]))
            am_bc.append(t)
        kb1_sb = persist.tile([P, C1 // P], dt); nc.sync.dma_start(kb1_sb[:], kb1[:])
        kb2_sb = persist.tile([P, HQ // P], dt); nc.sync.dma_start(kb2_sb[:], kb2[:])
        vb1_sb = persist.tile([P, C1 // P], dt); nc.sync.dma_start(vb1_sb[:], vb1[:])
        vb2_sb = persist.tile([P, HQ // P], dt); nc.sync.dma_start(vb2_sb[:], vb2[:])
        lng_sb = persist.tile([P, NLAYERS], dt); nc.sync.dma_start(lng_sb[:], lngneg[:])
        lnb_sb = persist.tile([P, NLAYERS], dt); nc.sync.dma_start(lnb_sb[:], lnb[:])
        ones_sb = consts.tile([P, P], dt)
        nc.vector.memset(ones_sb[:], 1.0)
        eps_sb = consts.tile([P, 1], dt)
        nc.vector.memset(eps_sb[:], EPS)

        # ---------------- Phase A ----------------
        actx = ExitStack()
        a_in = actx.enter_context(tc.tile_pool(name="a_in", bufs=4))
        qeT_t = []
        for k in range(4):
            t = a_in.tile([P, TOK], dt, tag="qeT")
            nc.sync.dma_start(t[:], qeT[k * P:(k + 1) * P, :])
            qeT_t.append(t)

        h1_pool = actx.enter_context(tc.tile_pool(name="h1", bufs=16))
        w1_pool = actx.enter_context(tc.tile_pool(name="w1s", bufs=8))
        w2_pool = actx.enter_context(tc.tile_pool(name="w2s", bufs=8))
        wexp_pool = actx.enter_context(tc.tile_pool(name="wexp", bufs=8))
        tmp_pool = actx.enter_context(tc.tile_pool(name="wexp_tmp", bufs=3))
        v_pool = actx.enter_context(tc.tile_pool(name="vp", bufs=8))
        ps_a = actx.enter_context(tc.tile_pool(name="ps_a", bufs=4, space="PSUM"))
        pool_small = actx.enter_context(tc.tile_pool(name="small", bufs=8))

        qq_sb = {}   # (s, hq_tile) -> [P, 1]

        for branch in range(2):  # 0 = k (wexp), 1 = v
            W1, b1_sb = (kW1, kb1_sb) if branch == 0 else (vW1, vb1_sb)
            W2, b2_sb = (kW2, kb2_sb) if branch == 0 else (vW2, vb2_sb)
            # MM1: h1T[c, tok] = relu(W1.T @ qeT + b1)
            h1_t = []
            for m in range(C1 // P):        # 16 c-tiles
                ps = ps_a.tile([P, TOK], dt, tag="psA")
                for k in range(4):
                    w = w1_pool.tile([P, P], dt, tag="w1")
                    nc.sync.dma_start(w[:], W1[k * P:(k + 1) * P, m * P:(m + 1) * P])
                    for n in range(2):
                        nc.tensor.matmul(ps[:, n * 512:(n + 1) * 512], w[:],
                                         qeT_t[k][:, n * 512:(n + 1) * 512],
                                         start=(k == 0), stop=(k == 3))
                h = h1_pool.tile([P, TOK], dt, tag="h1")
                nc.scalar.activation(h[:], ps[:], AF.Relu, bias=b1_sb[:, m:m + 1])
                h1_t.append(h)
            # MM2: zT[hq, tok] = W2.T @ h1T  (evict: tanh(+b2), then exp/mask)
            for m in range(HQ // P):        # 8 hq-tiles
                ps = ps_a.tile([P, TOK], dt, tag="psA")
                for k in range(C1 // P):    # 16
                    w = w2_pool.tile([P, P], dt, tag="w2")
                    nc.sync.dma_start(w[:], W2[k * P:(k + 1) * P, m * P:(m + 1) * P])
                    for n in range(2):
                        nc.tensor.matmul(ps[:, n * 512:(n + 1) * 512], w[:],
                                         h1_t[k][:, n * 512:(n + 1) * 512],
                                         start=(k == 0), stop=(k == 15))
                if branch == 0:
                    th = tmp_pool.tile([P, TOK], dt, tag="tmp")
                    nc.scalar.activation(th[:], ps[:], AF.Tanh, bias=b2_sb[:, m:m + 1])
                    ex = tmp_pool.tile([P, TOK], dt, tag="tmp")
                    nc.scalar.activation(ex[:], th[:], AF.Exp)
                    we = wexp_pool.tile([P, TOK], dt, tag="wexp")
                    nc.vector.tensor_mul(we[:], ex[:], qm_bc[:])
                    qq_sb[("wexp", m)] = we
                else:
                    vv = v_pool.tile([P, TOK], dt, tag="v")
                    nc.scalar.activation(vv[:], ps[:], AF.Tanh, bias=b2_sb[:, m:m + 1])
                    qq_sb[("v", m)] = vv

        # pooling: per hq-tile, per sample
        for m in range(HQ // P):
            we, vv = qq_sb[("wexp", m)], qq_sb[("v", m)]
            wv = tmp_pool.tile([P, TOK], dt, tag="tmp")
            nc.vector.tensor_mul(wv[:], we[:], vv[:])
            for s in range(SPC):
                num = pool_small.tile([P, 1], dt, tag="nd")
                den = pool_small.tile([P, 1], dt, tag="nd")
                nc.vector.reduce_sum(num[:], wv[:, s * Lq:(s + 1) * Lq],
                                     axis=mybir.AxisListType.X)
                nc.vector.reduce_sum(den[:], we[:, s * Lq:(s + 1) * Lq],
                                     axis=mybir.AxisListType.X)
                rd = pool_small.tile([P, 1], dt, tag="nd")
                nc.vector.reciprocal_approx_fast(out=rd[:], in_=den[:])
                qq = pool_small.tile([P, 1], dt, tag="nd")
                nc.vector.tensor_mul(qq[:], num[:], rd[:])
                # hq-tile m covers h = m//4, q in [128*(m%4), +128)
                h, q0 = divmod(m, 4)
                nc.sync.dma_start(
                    ag_in[2 * s + h:2 * s + h + 1, q0 * P:(q0 + 1) * P]
                    .rearrange("one q -> q one"),
                    qq[:])

        nc.gpsimd.collective_compute(
            "AllGather", mybir.AluOpType.bypass, replica_groups=groups,
            ins=[ag_in.opt()], outs=[qq_all.opt()])
        if debug:
            nc.sync.dma_start(dbg["qq"][:], qq_all[:])
        actx.close()

        # ---------------- Phase B ----------------
        bctx = ExitStack()
        bpool = bctx.enter_context(tc.tile_pool(name="bpool", bufs=4))
        gf_pool = bctx.enter_context(tc.tile_pool(name="gf", bufs=8))
        ps_b = bctx.enter_context(tc.tile_pool(name="ps_b", bufs=4, space="PSUM"))

        # qqT [512, 32] via PE transpose of qq_all [32, 512]
        qq_sb_full = bpool.tile([4 * NC, Q], dt, tag="qqsb")
        nc.sync.dma_start(qq_sb_full[:], qq_all[:])
        ident = consts.tile([P, P], dt)
        from concourse.masks import make_identity
        make_identity(nc, ident[:])
        qqT_t = []
        for k in range(4):
            pst = ps_b.tile([P, 4 * NC], dt, tag="psT")
            nc.tensor.transpose(pst[:], qq_sb_full[:, k * P:(k + 1) * P],
                                ident[:4 * NC, :4 * NC])
            qt = bpool.tile([P, 4 * NC], dt, tag="qqT")
            nc.scalar.activation(qt[:], pst[:], AF.Copy)
            qqT_t.append(qt)

        NCHUNK = 1024
        nchunks = (SLICE_COLS + NCHUNK - 1) // NCHUNK   # 17 (last = 120)
        for ci in range(nchunks):
            c0 = ci * NCHUNK
            cw = min(NCHUNK, SLICE_COLS - c0)
            gts = []
            for k in range(4):
                gt = gf_pool.tile([P, NCHUNK], dt, tag="gfw")
                nc.sync.dma_start(gt[:, :cw], gfW[k * P:(k + 1) * P, c0:c0 + cw])
                gts.append(gt)
            gbb = gf_pool.tile([4 * NC, NCHUNK], dt, tag="gfb")
            nc.sync.dma_start(gbb[:, :cw],
                              gfb[0:1, c0:c0 + cw].to_broadcast([4 * NC, cw]))
            for nsub in range(0, cw, 512):
                nw = min(512, cw - nsub)
                ps = ps_b.tile([4 * NC, 512], dt, tag="psB")
                for k in range(4):
                    nc.tensor.matmul(ps[:, :nw], qqT_t[k][:],
                                     gts[k][:, nsub:nsub + nw],
                                     start=(k == 0), stop=(k == 3))
                pv = bpool.tile([4 * NC, 512], dt, tag="pv")
                nc.vector.tensor_add(pv[:, :nw], ps[:, :nw],
                                     gbb[:, nsub:nsub + nw])
                nc.sync.dma_start(a2a_in[:, c0 + nsub:c0 + nsub + nw], pv[:, :nw])

        nc.gpsimd.collective_compute(
            "AllToAll", mybir.AluOpType.bypass, replica_groups=groups,
            ins=[a2a_in.opt()], outs=[a2a_out.opt()])
        if debug:
            nc.sync.dma_start(dbg["pvec"][:], a2a_in[:])
        bctx.close()

        # ---------------- Phase C ----------------
        cctx = ExitStack()
        ah_pool = cctx.enter_context(tc.tile_pool(name="ah", bufs=4))
        ahT_t = []
        for k in range(4):
            t = ah_pool.tile([P, TOK], dt, tag="ahT")
            nc.sync.dma_start(t[:], ahT[k * P:(k + 1) * P, :])
            ahT_t.append(t)

        wc_pool = cctx.enter_context(tc.tile_pool(name="wc", bufs=4))
        x_pool = cctx.enter_context(tc.tile_pool(name="xp", bufs=8))
        st_pool = cctx.enter_context(tc.tile_pool(name="st", bufs=2))
        ps_conv = cctx.enter_context(tc.tile_pool(name="ps_c", bufs=2, space="PSUM"))
        ps_stat = cctx.enter_context(tc.tile_pool(name="ps_s", bufs=2, space="PSUM"))
        ps_feat = cctx.enter_context(tc.tile_pool(name="ps_f", bufs=2, space="PSUM"))

        a2a_v = a2a_out[:].rearrange("(i r) c -> i r c", i=NC)
        of_acc = persist.tile([F, RPC], dt)
        nc.vector.memset(of_acc[:], 0.0)

        x_cur = [None] * RPC
        for L in range(NLAYERS):
            Dp, E = IN_DIMS[L], E_LIST[L]
            o = LAYER_OFF[L]
            Lout = La - L
            dpc = Dp // NC                     # w rows per core piece
            # conv weight tiles, all based at partition 0:
            #   L0: 4 k-tiles of 128 rows;  L>=1: 2 half-tiles of Ep rows
            chunk_rows = P if L == 0 else IN_DIMS[L] // 2
            nkt = Dp // chunk_rows
            w4_t = []
            for kk in range(nkt):
                wt = wc_pool.tile([chunk_rows, RPC * E], dt, tag=f"w4_{min(L, 3)}")
                w4_t.append(wt)
            for i in range(NC):
                d0 = i * dpc
                kk, prow = divmod(d0, chunk_rows)
                src = a2a_v[i, :, o["w"]:o["w"] + o["w_len"]]
                nc.sync.dma_start(
                    w4_t[kk][prow:prow + dpc, :].rearrange("d (r e) -> d r e", e=E),
                    src.rearrange("r (d e) -> d r e", e=E))
            b4 = wc_pool.tile([P, RPC], dt, tag="b4")
            for i in range(NC):
                e0 = i * (E // NC)
                nc.sync.dma_start(
                    b4[e0:e0 + E // NC, :],
                    a2a_v[i, :, o["b"]:o["b"] + o["b_len"]].rearrange("r e -> e r"))
            fw4 = wc_pool.tile([P, RPC * F], dt, tag="fw4")
            for i in range(NC):
                e0 = i * (E // NC)
                nc.sync.dma_start(
                    fw4[e0:e0 + E // NC, :].rearrange("e (r f) -> e r f", f=F),
                    a2a_v[i, :, o["fw"]:o["fw"] + o["fw_len"]].rearrange(
                        "r (e f) -> e r f", f=F))
            fb4 = wc_pool.tile([P, RPC], dt, tag="fb4")
            for i in range(NC):
                f0 = i * (F // NC)
                nc.sync.dma_start(
                    fb4[f0:f0 + F // NC, :],
                    a2a_v[i, :, o["fb"]:o["fb"] + o["fb_len"]].rearrange("r f -> f r"))

            for r in range(RPC):
                s = r // H
                ps = ps_conv.tile([P, 512], dt, tag="psc")
                if L == 0:
                    for k in range(4):
                        nc.tensor.matmul(
                            ps[:E, :Lout], w4_t[k][:, r * E:(r + 1) * E],
                            ahT_t[k][:, s * La:s * La + Lout],
                            start=(k == 0), stop=(k == 3))
                else:
                    Ep = E_LIST[L - 1]
                    xp = x_cur[r]
                    for half in range(2):
                        nc.tensor.matmul(
                            ps[:E, :Lout],
                            w4_t[half][:Ep, r * E:(r + 1) * E],
                            xp[:Ep, half:half + Lout],
                            start=(half == 0), stop=(half == 1))
                # relu(+b) then mask
                xr = st_pool.tile([P, 512], dt, tag="xr")
                nc.scalar.activation(xr[:E, :Lout], ps[:E, :Lout], AF.Relu,
                                     bias=b4[:E, r:r + 1])
                xm = x_pool.tile([P, 512], dt, tag="xm")
                nc.vector.tensor_tensor(
                    xm[:E, :Lout], xr[:E, :Lout],
                    am_bc[s][:E, :Lout], ALU.mult)
                # stats via all-ones matmul (broadcast sums)
                sq = st_pool.tile([P, 512], dt, tag="sq")
                nc.vector.tensor_mul(sq[:E, :Lout], xm[:E, :Lout], xm[:E, :Lout])
                s1 = ps_stat.tile([P, 512], dt, tag="s1")
                s2 = ps_stat.tile([P, 512], dt, tag="s2")
                nc.tensor.matmul(s1[:E, :Lout], ones_sb[:E, :E], xm[:E, :Lout],
                                 start=True, stop=True)
                nc.tensor.matmul(s2[:E, :Lout], ones_sb[:E, :E], sq[:E, :Lout],
                                 start=True, stop=True)
                # mu = s1/E (to SBUF); var = s2/E - mu^2 ; sd = sqrt(var + EPS)
                mu = st_pool.tile([P, 512], dt, tag="mu")
                nc.scalar.activation(mu[:E, :Lout], s1[:E, :Lout], AF.Copy,
                                     scale=1.0 / E)
                t1 = st_pool.tile([P, 512], dt, tag="t1")
                nc.vector.scalar_tensor_tensor(
                    t1[:E, :Lout], s1[:E, :Lout], 1.0 / E, mu[:E, :Lout],
                    ALU.mult, ALU.mult)
                vr = st_pool.tile([P, 512], dt, tag="vr")
                nc.vector.scalar_tensor_tensor(
                    vr[:E, :Lout], s2[:E, :Lout], 1.0 / E, t1[:E, :Lout],
                    ALU.mult, ALU.subtract)
                sd = st_pool.tile([P, 512], dt, tag="sd")
                nc.scalar.activation(sd[:E, :Lout], vr[:E, :Lout], AF.Sqrt,
                                     bias=eps_sb[:E, :])
                inv = st_pool.tile([P, 512], dt, tag="inv")
                nc.vector.reciprocal_approx_fast(out=inv[:E, :Lout], in_=sd[:E, :Lout])
                # tm = mu - x ;  u = tm * (-g) * inv
                tm = st_pool.tile([P, 512], dt, tag="tm")
                nc.vector.tensor_sub(tm[:E, :Lout], mu[:E, :Lout], xm[:E, :Lout])
                u = st_pool.tile([P, 512], dt, tag="u")
                nc.vector.scalar_tensor_tensor(
                    u[:E, :Lout], tm[:E, :Lout], lng_sb[:E, L:L + 1],
                    inv[:E, :Lout], ALU.mult, ALU.mult)
                xn = x_pool.tile([P, 512], dt, tag="xn")
                nc.scalar.activation(xn[:E, :Lout], u[:E, :Lout], AF.Identity,
                                     bias=lnb_sb[:E, L:L + 1])
                x_cur[r] = xn
                # maxpool + feat
                mx = st_pool.tile([P, 1], dt, tag="mx")
                nc.vector.reduce_max(mx[:E, :], xn[:E, :Lout],
                                     axis=mybir.AxisListType.X)
                pf = ps_feat.tile([P, 1], dt, tag="psf")
                nc.tensor.matmul(pf[:F, :], fw4[:E, r * F:(r + 1) * F], mx[:E, :],
                                 start=True, stop=True)
                off = st_pool.tile([P, 1], dt, tag="of")
                nc.scalar.activation(off[:F, :], pf[:F, :], AF.Relu,
                                     bias=fb4[:F, r:r + 1])
                nc.vector.tensor_add(of_acc[:, r:r + 1], of_acc[:, r:r + 1],
                                     off[:F, :])

        # a_rep rows r -> arep_in [4, 128]
        nc.sync.dma_start(
            arep_in[:].rearrange("r f -> f r"), of_acc[:, :])

        nc.gpsimd.collective_compute(
            "AllGather", mybir.AluOpType.bypass, replica_groups=groups,
            ins=[arep_in.opt()], outs=[arep_all.opt()])
        if debug:
            nc.sync.dma_start(dbg["arep"][:], arep_all[:])
        cctx.close()

        # ---------------- classifier ----------------
        cls_pool = ctx.enter_context(tc.tile_pool(name="cls", bufs=8))
        ps_cls = ctx.enter_context(tc.tile_pool(name="ps_cls", bufs=4, space="PSUM"))

        # a_repT [256, 16] (2 k-tiles): partition p=f, row h*128+f; addr = 256b+128h+f
        arT = []
        for h in range(H):
            t = cls_pool.tile([P, B], dt, tag="arT")
            nc.sync.dma_start(
                t[:], arep_all[:].rearrange("(b h) f -> (h f) b", h=H)[
                    h * P:(h + 1) * P, :])
            arT.append(t)

        cw_sb = {}
        for name, apw, kt, mt in (("cW1", cW1, 2, 4), ("cW2", cW2, 4, 4),
                                  ("cW3", cW3, 4, 1)):
            for k in range(kt):
                t = cls_pool.tile([P, apw.shape[1]], dt, tag=f"{name}k")
                nc.sync.dma_start(t[:], apw[k * P:(k + 1) * P, :])
                cw_sb[(name, k)] = t
        cb_sb = {}
        for name, apb in (("cb1", cb1), ("cg1", cg1), ("cbt1", cbt1),
                          ("cb2", cb2), ("cg2", cg2), ("cbt2", cbt2)):
            t = cls_pool.tile([P, 4], dt, tag="cbias")
            nc.sync.dma_start(t[:], apb[:])
            cb_sb[name] = t
        cb3_sb = cls_pool.tile([4, 1], dt, tag="cb3")
        nc.sync.dma_start(cb3_sb[:], cb3[:])

        def bn_layer(h_tiles, gname, btname):
            outs = []
            for m, ht in enumerate(h_tiles):
                mean = cls_pool.tile([P, 1], dt, tag="clsm")
                nc.vector.reduce_sum(mean[:], ht[:], axis=mybir.AxisListType.X)
                nc.vector.tensor_scalar(out=mean[:], in0=mean[:],
                                        scalar1=1.0 / B, scalar2=None,
                                        op0=ALU.mult)
                sq = cls_pool.tile([P, B], dt, tag="clssq")
                nc.vector.tensor_mul(sq[:], ht[:], ht[:])
                m2 = cls_pool.tile([P, 1], dt, tag="clsm")
                nc.vector.reduce_sum(m2[:], sq[:], axis=mybir.AxisListType.X)
                msq = cls_pool.tile([P, 1], dt, tag="clsm")
                nc.vector.tensor_mul(msq[:], mean[:], mean[:])
                var = cls_pool.tile([P, 1], dt, tag="clsm")
                nc.vector.scalar_tensor_tensor(var[:], m2[:], 1.0 / B, msq[:],
                                               ALU.mult, ALU.subtract)
                sd = cls_pool.tile([P, 1], dt, tag="clsm")
                nc.scalar.activation(sd[:], var[:], AF.Sqrt, bias=eps_sb[:])
                inv = cls_pool.tile([P, 1], dt, tag="clsm")
                nc.vector.reciprocal_approx_fast(out=inv[:], in_=sd[:])
                hn = cls_pool.tile([P, B], dt, tag="clsh")
                nc.vector.tensor_scalar(out=hn[:], in0=ht[:], scalar1=mean[:],
                                        scalar2=inv[:], op0=ALU.subtract,
                                        op1=ALU.mult)
                ho = cls_pool.tile([P, B], dt, tag="clsh")
                nc.vector.tensor_scalar(out=ho[:], in0=hn[:],
                                        scalar1=cb_sb[gname][:, m:m + 1],
                                        scalar2=cb_sb[btname][:, m:m + 1],
                                        op0=ALU.mult, op1=ALU.add)
                outs.append(ho)
            return outs

        # layer 1
        h1_tiles = []
        for m in range(4):
            ps = ps_cls.tile([P, B], dt, tag="pscls")
            for k in range(2):
                nc.tensor.matmul(ps[:], cw_sb[("cW1", k)][:, m * P:(m + 1) * P],
                                 arT[k][:], start=(k == 0), stop=(k == 1))
            ht = cls_pool.tile([P, B], dt, tag="clsh")
            nc.scalar.activation(ht[:], ps[:], AF.Relu,
                                 bias=cb_sb["cb1"][:, m:m + 1])
            h1_tiles.append(ht)
        h1n = bn_layer(h1_tiles, "cg1", "cbt1")
        # layer 2
        h2_tiles = []
        for m in range(4):
            ps = ps_cls.tile([P, B], dt, tag="pscls")
            for k in range(4):
                nc.tensor.matmul(ps[:], cw_sb[("cW2", k)][:, m * P:(m + 1) * P],
                                 h1n[k][:], start=(k == 0), stop=(k == 3))
            ht = cls_pool.tile([P, B], dt, tag="clsh")
            nc.scalar.activation(ht[:], ps[:], AF.Relu,
                                 bias=cb_sb["cb2"][:, m:m + 1])
            h2_tiles.append(ht)
        h2n = bn_layer(h2_tiles, "cg2", "cbt2")
        # layer 3
        ps = ps_cls.tile([4, B], dt, tag="pscls3")
        for k in range(4):
            nc.tensor.matmul(ps[:], cw_sb[("cW3", k)][:], h2n[k][:],
                             start=(k == 0), stop=(k == 3))
        ho = cls_pool.tile([4, B], dt, tag="clsout")
        nc.scalar.activation(ho[:], ps[:], AF.Identity, bias=cb3_sb[:])
        nc.sync.dma_start(out_ap[:].rearrange("b j -> j b"), ho[:])

    nc.compile()
    return nc


# ---------------- host marshalling ----------------
def host_prep(q_embeddings, q_attention_mask, a_hidden, a_attention_mask, params):
    p = params
    qe = np.ascontiguousarray(np.asarray(q_embeddings, np.float32))
    ah = np.ascontiguousarray(np.asarray(a_hidden, np.float32))
    qm = np.asarray(q_attention_mask)
    am = np.asarray(a_attention_mask)

    def f32(x):
        return np.ascontiguousarray(np.asarray(x, np.float32))

    # per-core gf_W column slices
    gfW_sl, gfb_sl = [], []
    gfW_full = [f32(w) for w in p["gf_W"]]
    gfb_full = [f32(b) for b in p["gf_b"]]
    for i in range(NC):
        Ws, bs = [], []
        for L in range(NLAYERS):
            Dp, E = IN_DIMS[L], E_LIST[L]
            b0 = Dp * E
            fw0 = b0 + E
            fb0 = fw0 + E * F
            d_rows = np.arange(i * Dp // NC, (i + 1) * Dp // NC)
            w_cols = (d_rows[:, None] * E + np.arange(E)[None, :]).ravel()
            b_cols = b0 + np.arange(i * E // NC, (i + 1) * E // NC)
            e_rows = np.arange(i * E // NC, (i + 1) * E // NC)
            fw_cols = (fw0 + e_rows[:, None] * F + np.arange(F)[None, :]).ravel()
            fb_cols = fb0 + np.arange(i * F // NC, (i + 1) * F // NC)
            cols = np.concatenate([w_cols, b_cols, fw_cols, fb_cols])
            Ws.append(gfW_full[L][:, cols])
            bs.append(gfb_full[L][cols])
        gfW_sl.append(np.ascontiguousarray(np.concatenate(Ws, 1)))
        gfb_sl.append(np.ascontiguousarray(np.concatenate(bs)[None, :]))

    lngneg = np.zeros((P, NLAYERS), np.float32)
    lnb = np.zeros((P, NLAYERS), np.float32)
    for L in range(NLAYERS):
        lngneg[:E_LIST[L], L] = -f32(p["ln_g"][L])
        lnb[:E_LIST[L], L] = f32(p["ln_b"][L])

    def ptile(x, cols):
        return np.ascontiguousarray(f32(x).reshape(cols, P).T)

    common = {
        "kW1": f32(p["kW1"]), "kb1": ptile(p["kb1"], 16),
        "kW2": f32(p["kW2"]), "kb2": ptile(p["kb2"], 8),
        "vW1": f32(p["vW1"]), "vb1": ptile(p["vb1"], 16),
        "vW2": f32(p["vW2"]), "vb2": ptile(p["vb2"], 8),
        "lngneg": lngneg, "lnb": lnb,
        "cW1": f32(p["cW1"]), "cb1": ptile(p["cb1"], 4),
        "cg1": ptile(p["cg1"], 4), "cbt1": ptile(p["cbt1"], 4),
        "cW2": f32(p["cW2"]), "cb2": ptile(p["cb2"], 4),
        "cg2": ptile(p["cg2"], 4), "cbt2": ptile(p["cbt2"], 4),
        "cW3": f32(p["cW3"]), "cb3": f32(p["cb3"]).reshape(4, 1),
    }

    in_maps = []
    for i in range(NC):
        s0 = i * SPC
        in_maps.append({
            **common,
            "qeT": np.ascontiguousarray(qe[s0:s0 + SPC].reshape(TOK, D).T),
            "ahT": np.ascontiguousarray(ah[s0:s0 + SPC].reshape(TOK, D).T),
            "qmf": (qm[s0:s0 + SPC] > 0).astype(np.float32).reshape(1, TOK),
            "amf": (am[s0:s0 + SPC] > 0).astype(np.float32),
            "gfW": gfW_sl[i], "gfb": gfb_sl[i],
        })
    return in_maps


_NC_CACHE = {}


def _get_program(debug=False):
    key = bool(debug)
    if key not in _NC_CACHE:
        _NC_CACHE[key] = build_program(debug=key)
    return _NC_CACHE[key]


def kernel(q_embeddings, q_attention_mask, a_hidden, a_attention_mask, params,
           _debug=False, _return_results=False):
    nc = _get_program(debug=_debug)
    in_maps = host_prep(q_embeddings, q_attention_mask, a_hidden,
                        a_attention_mask, params)
    res = run_bass_kernel_spmd(nc, in_maps, list(range(NC)))
    if _return_results:
        return res
    return res.results[0]["out"]
